# revision 2
# baseline (speedup 1.0000x reference)
"""DLRM (embedding_lookup) Trainium2 Bass kernel.

Strategy: pure data parallelism over the batch. Each of the 8 NeuronCores
holds all 26 embedding tables (replicated in its HBM as one flattened
bf16 [26*200000, 64] tensor; host pre-biases indices by t*V) and
processes a 512-sample slice of the 4096 batch end-to-end. No
collectives; host shards inputs / concatenates outputs.

Key performance structure (vs the one-lookup-per-DMA baseline):
  * One indirect DMA per TABLE: offset AP [128, NT*L] = 80 indices per
    partition -> 10240 descriptors per instruction. SWDGE descriptor
    generation has ~1 us fixed cost per instruction, so batching 80x
    fewer instructions removes ~2 ms of serialized DGE time.
  * Tables are gathered in bf16 (host-converted). 128-byte descriptors
    halve HBM gather traffic vs f32; rel tolerance 2e-2 has ample room.
  * Pooling of the 20 lookups: 5 strided DVE adds per table over
    4-D views; first level converts bf16->f32, tree continues in f32.
  * PE transposes pooled [128,64] -> [64,128] into feature-major
    featT [1792, 512] (64 zero pad rows).
  * Top-MLP first layer (1728->512, the big GEMM) is accumulated
    chunk-by-chunk in 4 persistent PSUM banks, interleaved into the
    gather stream as each 128-row feature chunk completes, so only the
    512->256->1 tail runs after the last gather.
  * Bottom MLP (13->512->256->64) runs up front, feature-major, f32.
"""

import numpy as np
import ml_dtypes

import concourse.bass as bass
import concourse.mybir as mybir
import concourse.tile as tile
from concourse import bacc
from concourse.bass_utils import run_bass_kernel_spmd
from concourse.masks import make_identity

F32 = mybir.dt.float32
BF16 = mybir.dt.bfloat16
I32 = mybir.dt.int32
AF = mybir.ActivationFunctionType

B = 4096
T = 26
V = 200000
L = 20
D = 64
DENSE = 13
NCORES = 8


def build_bass(T=T, V=V, NT=4, L=L, D=D):
    Bc = NT * 128
    NCH = (T + 2) // 2
    pad_rows = NCH * 128 - D * (T + 1)
    assert Bc <= 512

    nc = bacc.Bacc(
        "TRN2", target_bir_lowering=False, debug=False,
        enable_asserts=False, num_devices=1,
    )

    tables = nc.dram_tensor("tables", [T * V, D], BF16, kind="ExternalInput")
    idx = nc.dram_tensor("idx", [128, T * NT * L], I32, kind="ExternalInput")
    xdt = nc.dram_tensor("xdt", [128, Bc], F32, kind="ExternalInput")
    wb0 = nc.dram_tensor("wb0", [128, 512], F32, kind="ExternalInput")
    wb1 = nc.dram_tensor("wb1", [128, 1024], F32, kind="ExternalInput")
    wb2 = nc.dram_tensor("wb2", [128, 128], F32, kind="ExternalInput")
    wt0 = nc.dram_tensor("wt0", [128, NCH * 512], F32, kind="ExternalInput")
    wt1 = nc.dram_tensor("wt1", [128, 1024], F32, kind="ExternalInput")
    wt2 = nc.dram_tensor("wt2", [128, 2], F32, kind="ExternalInput")
    bb0 = nc.dram_tensor("bb0", [128, 4], F32, kind="ExternalInput")
    bb1 = nc.dram_tensor("bb1", [128, 2], F32, kind="ExternalInput")
    bb2 = nc.dram_tensor("bb2", [64, 1], F32, kind="ExternalInput")
    tb0 = nc.dram_tensor("tb0", [128, 4], F32, kind="ExternalInput")
    tb1 = nc.dram_tensor("tb1", [128, 2], F32, kind="ExternalInput")
    tb2 = nc.dram_tensor("tb2", [1, 1], F32, kind="ExternalInput")
    y = nc.dram_tensor("y", [1, Bc], F32, kind="ExternalOutput")

    with tile.TileContext(nc) as tc:
        with (
            tc.tile_pool(name="const", bufs=1) as cpool,
            tc.tile_pool(name="acts", bufs=1) as apool,
            tc.tile_pool(name="stage", bufs=3) as spool,
            tc.tile_pool(name="pool32", bufs=3) as ppool,
            tc.tile_pool(name="mm", bufs=2, space="PSUM") as mmpool,
            tc.tile_pool(name="tp", bufs=2, space="PSUM") as tppool,
            tc.tile_pool(name="zacc", bufs=1, space="PSUM") as zpool,
        ):
            ident = cpool.tile([128, 128], F32)
            make_identity(nc, ident[:])

            def load(dram, shape, dtype=F32):
                t = cpool.tile(shape, dtype, tag=dram.name)
                nc.sync.dma_start(out=t[:], in_=dram.ap())
                return t

            idx_sb = load(idx, [128, T * NT * L], I32)
            xdt_sb = load(xdt, [128, Bc])
            wb0_sb = load(wb0, [128, 512])
            wb1_sb = load(wb1, [128, 1024])
            wb2_sb = load(wb2, [128, 128])
            wt0_sb = load(wt0, [128, NCH * 512])
            wt1_sb = load(wt1, [128, 1024])
            wt2_sb = load(wt2, [128, 2])
            bb0_sb = load(bb0, [128, 4])
            bb1_sb = load(bb1, [128, 2])
            bb2_sb = load(bb2, [64, 1])
            tb0_sb = load(tb0, [128, 4])
            tb1_sb = load(tb1, [128, 2])
            tb2_sb = load(tb2, [1, 1])

            featT = apool.tile([128, NCH * Bc], F32)
            if pad_rows:
                nc.vector.memset(featT[128 - pad_rows:, (NCH - 1) * Bc:], 0.0)

            # persistent PSUM accumulators for the 4 output blocks of the
            # top-MLP first layer; accumulated chunk-by-chunk during the
            # gather stream
            z_ps = [zpool.tile([128, 512], F32, name=f"zacc{o}")
                    for o in range(4)]

            # ---------------- bottom MLP ----------------
            h0 = apool.tile([128, 4 * Bc], F32)
            for o in range(4):
                ps = mmpool.tile([128, 512], F32)
                nc.tensor.matmul(
                    out=ps[:, :Bc], lhsT=wb0_sb[:, o * 128:(o + 1) * 128],
                    rhs=xdt_sb[:], start=True, stop=True)
                nc.scalar.activation(
                    out=h0[:, o * Bc:(o + 1) * Bc], in_=ps[:, :Bc],
                    func=AF.Relu, bias=bb0_sb[:, o:o + 1])
            h1 = apool.tile([128, 2 * Bc], F32)
            for o in range(2):
                ps = mmpool.tile([128, 512], F32)
                for k in range(4):
                    nc.tensor.matmul(
                        out=ps[:, :Bc],
                        lhsT=wb1_sb[:, k * 256 + o * 128:k * 256 + o * 128 + 128],
                        rhs=h0[:, k * Bc:(k + 1) * Bc],
                        start=(k == 0), stop=(k == 3))
                nc.scalar.activation(
                    out=h1[:, o * Bc:(o + 1) * Bc], in_=ps[:, :Bc],
                    func=AF.Relu, bias=bb1_sb[:, o:o + 1])
            ps = mmpool.tile([128, 512], F32)
            for k in range(2):
                nc.tensor.matmul(
                    out=ps[:64, :Bc], lhsT=wb2_sb[:, k * 64:(k + 1) * 64],
                    rhs=h1[:, k * Bc:(k + 1) * Bc],
                    start=(k == 0), stop=(k == 1))
            nc.scalar.activation(
                out=featT[0:64, 0:Bc], in_=ps[:64, :Bc],
                func=AF.Relu, bias=bb2_sb[:, 0:1])

            # ------------- embedding gather + pool + top-MLP layer 0 -------------
            def chunk_matmul(c):
                for o in range(4):
                    nc.tensor.matmul(
                        out=z_ps[o][:, :Bc],
                        lhsT=wt0_sb[:, c * 512 + o * 128:c * 512 + o * 128 + 128],
                        rhs=featT[:, c * Bc:(c + 1) * Bc],
                        start=(c == 0), stop=(c == NCH - 1))

            for t in range(T):
                st = spool.tile([128, NT, L, D], BF16, tag="stage")
                cb = t * NT * L
                nc.gpsimd.indirect_dma_start(
                    out=st.rearrange("p a l d -> p (a l) d"),
                    out_offset=None,
                    in_=tables.ap(),
                    in_offset=bass.IndirectOffsetOnAxis(
                        ap=idx_sb[:, cb:cb + NT * L], axis=0),
                )
                # pooling add-tree over the 20 lookups; level 1 converts
                # bf16 -> f32, remaining levels in f32 in place
                p32 = ppool.tile([128, NT, L // 2, D], F32, tag="p32")
                nc.vector.tensor_add(
                    out=p32[:, :, :, :], in0=st[:, :, 0:10, :],
                    in1=st[:, :, 10:20, :])
                nc.vector.tensor_add(
                    out=p32[:, :, 0:5, :], in0=p32[:, :, 0:5, :],
                    in1=p32[:, :, 5:10, :])
                nc.vector.tensor_add(
                    out=p32[:, :, 0:2, :], in0=p32[:, :, 0:2, :],
                    in1=p32[:, :, 2:4, :])
                nc.vector.tensor_add(
                    out=p32[:, :, 0:1, :], in0=p32[:, :, 0:1, :],
                    in1=p32[:, :, 1:2, :])
                nc.vector.tensor_add(
                    out=p32[:, :, 0:1, :], in0=p32[:, :, 0:1, :],
                    in1=p32[:, :, 4:5, :])
                c = (t + 1) // 2
                off = 64 * ((t + 1) % 2)
                for j in range(NT):
                    pst = tppool.tile([64, 128], F32, tag="tp")
                    nc.tensor.transpose(
                        out=pst[:], in_=p32[:, j, 0, :], identity=ident[:])
                    nc.scalar.copy(
                        out=featT[off:off + 64,
                                  c * Bc + j * 128:c * Bc + (j + 1) * 128],
                        in_=pst[:])
                # feature chunk c is complete: chunk 0 after table 0 (+bottom
                # MLP), chunk c>=1 after table 2c; the final chunk's upper 64
                # rows are the zero pad
                if t == 0:
                    chunk_matmul(0)
                elif t % 2 == 0:
                    chunk_matmul(t // 2)
                elif t == T - 1:
                    chunk_matmul((t + 1) // 2)

            # ---------------- top MLP tail ----------------
            z0 = apool.tile([128, 4 * Bc], F32)
            for o in range(4):
                nc.scalar.activation(
                    out=z0[:, o * Bc:(o + 1) * Bc], in_=z_ps[o][:, :Bc],
                    func=AF.Relu, bias=tb0_sb[:, o:o + 1])
            z1 = apool.tile([128, 2 * Bc], F32)
            for o in range(2):
                ps = mmpool.tile([128, 512], F32)
                for k in range(4):
                    nc.tensor.matmul(
                        out=ps[:, :Bc],
                        lhsT=wt1_sb[:, k * 256 + o * 128:k * 256 + o * 128 + 128],
                        rhs=z0[:, k * Bc:(k + 1) * Bc],
                        start=(k == 0), stop=(k == 3))
                nc.scalar.activation(
                    out=z1[:, o * Bc:(o + 1) * Bc], in_=ps[:, :Bc],
                    func=AF.Relu, bias=tb1_sb[:, o:o + 1])
            ps = mmpool.tile([128, 512], F32)
            for k in range(2):
                nc.tensor.matmul(
                    out=ps[0:1, :Bc], lhsT=wt2_sb[:, k:k + 1],
                    rhs=z1[:, k * Bc:(k + 1) * Bc],
                    start=(k == 0), stop=(k == 1))
            ysb = apool.tile([1, Bc], F32)
            nc.scalar.activation(
                out=ysb[:], in_=ps[0:1, :Bc],
                func=AF.Sigmoid, bias=tb2_sb[0:1, 0:1])
            nc.sync.dma_start(out=y.ap(), in_=ysb[:])

    nc.compile()
    return nc


def pack_weights(inp, T=T, D=D):
    NCH = (T + 2) // 2
    f32 = np.float32

    def kchunks(wT, K, M):
        return np.ascontiguousarray(
            wT.reshape(K // 128, 128, M).transpose(1, 0, 2).reshape(128, -1)
        ).astype(f32, copy=False)

    wb0 = np.zeros((128, 512), f32)
    wb0[:DENSE] = inp["bw0"].T
    wb1 = kchunks(np.ascontiguousarray(inp["bw1"].T), 512, 256)
    wb2 = kchunks(np.ascontiguousarray(inp["bw2"].T), 256, 64)
    feat_in = D * (1 + T)
    wt0p = np.zeros((NCH * 128, 512), f32)
    wt0p[:feat_in] = inp["tw0"].T
    wt0 = kchunks(wt0p, NCH * 128, 512)
    wt1 = kchunks(np.ascontiguousarray(inp["tw1"].T), 512, 256)
    wt2 = kchunks(np.ascontiguousarray(inp["tw2"].T), 256, 1)
    return dict(
        wb0=wb0, wb1=wb1, wb2=wb2, wt0=wt0, wt1=wt1, wt2=wt2,
        bb0=np.ascontiguousarray(inp["bb0"].reshape(4, 128).T).astype(f32),
        bb1=np.ascontiguousarray(inp["bb1"].reshape(2, 128).T).astype(f32),
        bb2=inp["bb2"].reshape(64, 1).astype(f32),
        tb0=np.ascontiguousarray(inp["tb0"].reshape(4, 128).T).astype(f32),
        tb1=np.ascontiguousarray(inp["tb1"].reshape(2, 128).T).astype(f32),
        tb2=inp["tb2"].reshape(1, 1).astype(f32),
    )


def pack_core(x_dense, x_indices, c, Bc, NT, T=T, V=V, L=L):
    sl = slice(c * Bc, (c + 1) * Bc)
    xdt = np.zeros((128, Bc), np.float32)
    xdt[:DENSE] = x_dense[sl].T
    idx = x_indices[:, sl, :].astype(np.int32)           # [T, Bc, L]
    idx += (np.arange(T, dtype=np.int32) * V)[:, None, None]
    idxp = np.ascontiguousarray(
        idx.reshape(T, NT, 128, L).transpose(2, 0, 1, 3).reshape(128, T * NT * L)
    )
    return xdt, idxp


_NC_CACHE = {}


def _get_nc():
    if "nc" not in _NC_CACHE:
        _NC_CACHE["nc"] = build_bass()
    return _NC_CACHE["nc"]


def run(inputs, trace=False, **run_kwargs):
    nc = _get_nc()
    NT = 4
    Bc = NT * 128
    shared = pack_weights(inputs)
    tables_flat = np.ascontiguousarray(
        np.asarray(inputs["tables"], dtype=np.float32).reshape(T * V, D)
    ).astype(ml_dtypes.bfloat16)
    x_dense = np.asarray(inputs["x_dense"], dtype=np.float32)
    x_indices = np.asarray(inputs["x_indices"])
    in_maps = []
    for c in range(NCORES):
        xdt, idxp = pack_core(x_dense, x_indices, c, Bc, NT)
        m = dict(shared)
        m["tables"] = tables_flat
        m["xdt"] = xdt
        m["idx"] = idxp
        in_maps.append(m)
    res = run_bass_kernel_spmd(
        nc, in_maps, core_ids=list(range(NCORES)), trace=trace, **run_kwargs)
    yv = np.concatenate([res.results[c]["y"][0] for c in range(NCORES)])
    return yv.reshape(B, 1).astype(np.float32), res


def kernel(**inputs):
    return run(inputs)[0]


# revision 3
# speedup vs baseline: 1.9876x; 1.9876x over previous
"""DLRM (embedding_lookup) Trainium2 Bass kernel.

Strategy: pure data parallelism over the batch. Each of the 8 NeuronCores
holds all 26 embedding tables (replicated in its HBM as one flattened
bf16 [26*200000, 64] tensor; host pre-biases indices by t*V) and
processes a 512-sample slice of the 4096 batch end-to-end. No
collectives; host shards inputs / concatenates outputs.

Performance structure (vs the f32 baseline):
  * Tables gathered in bf16: 128-byte descriptors halve HBM gather
    traffic/time (rel tolerance 2e-2 has ~100x slack).
  * Indirect gathers are issued one [128,1] offset column at a time
    (the indirect1d ucode's only correct mode), but round-robined over
    the 4 SWDGE queues so descriptor generation can proceed in
    parallel Q7 contexts instead of serializing on one.
  * Pooling: 5 strided DVE adds per (table, tile); first level converts
    bf16->f32, tree continues in f32.
  * PE transposes pooled [128,64] -> [64,128] into feature-major
    featT [1792, 512] (64 zero pad rows).
  * Top-MLP first layer (1728->512, the big GEMM) accumulates
    chunk-by-chunk in 4 persistent PSUM banks, interleaved into the
    gather stream as each 128-row feature chunk completes; only the
    512->256->1 tail runs after the last gather.
"""

import numpy as np
import ml_dtypes

import concourse.bass as bass
import concourse.mybir as mybir
import concourse.tile as tile
from concourse import bacc
from concourse.bass_utils import run_bass_kernel_spmd
from concourse.masks import make_identity

F32 = mybir.dt.float32
BF16 = mybir.dt.bfloat16
I32 = mybir.dt.int32
AF = mybir.ActivationFunctionType

B = 4096
T = 26
V = 200000
L = 20
D = 64
DENSE = 13
NCORES = 8
NQ = 4  # SWDGE queues to round-robin indirect gathers over


def _indirect_gather(nc, out, in_, offset_col, queue_num):
    """nc.gpsimd.indirect_dma_start, parameterized by SWDGE queue."""
    g = nc.gpsimd
    out_ap = g.lower_ap_dma(out, for_indirect_dma=True)
    in_ap = g.lower_ap_dma(in_, for_indirect_dma=True)
    assert len(in_ap) == 1 and len(out_ap) == 1
    off_ap = g.lower_ap_dma(offset_col)
    assert len(off_ap) == 1
    in_ap.append(off_ap[0])
    ap_shape = in_.shape
    coef = 1
    for i in range(1, len(ap_shape)):
        coef *= ap_shape[i]
    in_ap[0].dynamic_ap_info = mybir.DynamicAccessPatternInfo(
        c=0,
        actual_ap=out.ap,
        indirect_dim_max_index=ap_shape[0],
        offset_expr=[
            mybir.DynamicAccessPatternOffsetExpr(
                coef=coef,
                aff_expr=mybir.DynamicAccessPatternOffsetExprAffExpr(
                    kind="IndirectArgId", arg_id=1,
                ),
            )
        ],
    )
    qname = f"qPoolDynamic{queue_num or ''}"
    return g.add_instruction(
        mybir.InstDMACopy(
            name=nc.get_next_instruction_name(),
            queue=qname,
            mode="Copy",
            ins=in_ap,
            outs=out_ap,
            oob_is_err=True,
            cce_op=mybir.AluOpType.bypass,
        )
    )


def build_bass(T=T, V=V, NT=4, L=L, D=D, nq=NQ):
    Bc = NT * 128
    NCH = (T + 2) // 2
    pad_rows = NCH * 128 - D * (T + 1)
    assert Bc <= 512

    nc = bacc.Bacc(
        "TRN2", target_bir_lowering=False, debug=False,
        enable_asserts=False, num_devices=1, num_swdge_queues=nq,
    )

    tables = nc.dram_tensor("tables", [T * V, D], BF16, kind="ExternalInput")
    idx = nc.dram_tensor("idx", [128, T * NT * L], I32, kind="ExternalInput")
    xdt = nc.dram_tensor("xdt", [128, Bc], F32, kind="ExternalInput")
    wb0 = nc.dram_tensor("wb0", [128, 512], F32, kind="ExternalInput")
    wb1 = nc.dram_tensor("wb1", [128, 1024], F32, kind="ExternalInput")
    wb2 = nc.dram_tensor("wb2", [128, 128], F32, kind="ExternalInput")
    wt0 = nc.dram_tensor("wt0", [128, NCH * 512], F32, kind="ExternalInput")
    wt1 = nc.dram_tensor("wt1", [128, 1024], F32, kind="ExternalInput")
    wt2 = nc.dram_tensor("wt2", [128, 2], F32, kind="ExternalInput")
    bb0 = nc.dram_tensor("bb0", [128, 4], F32, kind="ExternalInput")
    bb1 = nc.dram_tensor("bb1", [128, 2], F32, kind="ExternalInput")
    bb2 = nc.dram_tensor("bb2", [64, 1], F32, kind="ExternalInput")
    tb0 = nc.dram_tensor("tb0", [128, 4], F32, kind="ExternalInput")
    tb1 = nc.dram_tensor("tb1", [128, 2], F32, kind="ExternalInput")
    tb2 = nc.dram_tensor("tb2", [1, 1], F32, kind="ExternalInput")
    y = nc.dram_tensor("y", [1, Bc], F32, kind="ExternalOutput")

    with tile.TileContext(nc) as tc:
        with (
            tc.tile_pool(name="const", bufs=1) as cpool,
            tc.tile_pool(name="acts", bufs=1) as apool,
            tc.tile_pool(name="stage", bufs=4) as spool,
            tc.tile_pool(name="pool32", bufs=3) as ppool,
            tc.tile_pool(name="mm", bufs=2, space="PSUM") as mmpool,
            tc.tile_pool(name="tp", bufs=2, space="PSUM") as tppool,
            tc.tile_pool(name="zacc", bufs=1, space="PSUM") as zpool,
        ):
            ident = cpool.tile([128, 128], F32)
            make_identity(nc, ident[:])

            def load(dram, shape, dtype=F32):
                t = cpool.tile(shape, dtype, tag=dram.name)
                nc.sync.dma_start(out=t[:], in_=dram.ap())
                return t

            idx_sb = load(idx, [128, T * NT * L], I32)
            xdt_sb = load(xdt, [128, Bc])
            wb0_sb = load(wb0, [128, 512])
            wb1_sb = load(wb1, [128, 1024])
            wb2_sb = load(wb2, [128, 128])
            wt0_sb = load(wt0, [128, NCH * 512])
            wt1_sb = load(wt1, [128, 1024])
            wt2_sb = load(wt2, [128, 2])
            bb0_sb = load(bb0, [128, 4])
            bb1_sb = load(bb1, [128, 2])
            bb2_sb = load(bb2, [64, 1])
            tb0_sb = load(tb0, [128, 4])
            tb1_sb = load(tb1, [128, 2])
            tb2_sb = load(tb2, [1, 1])

            featT = apool.tile([128, NCH * Bc], F32)
            if pad_rows:
                nc.vector.memset(featT[128 - pad_rows:, (NCH - 1) * Bc:], 0.0)

            # persistent PSUM accumulators for the top-MLP first layer
            z_ps = [zpool.tile([128, 512], F32, name=f"zacc{o}")
                    for o in range(4)]

            # ---------------- bottom MLP ----------------
            h0 = apool.tile([128, 4 * Bc], F32)
            for o in range(4):
                ps = mmpool.tile([128, 512], F32)
                nc.tensor.matmul(
                    out=ps[:, :Bc], lhsT=wb0_sb[:, o * 128:(o + 1) * 128],
                    rhs=xdt_sb[:], start=True, stop=True)
                nc.scalar.activation(
                    out=h0[:, o * Bc:(o + 1) * Bc], in_=ps[:, :Bc],
                    func=AF.Relu, bias=bb0_sb[:, o:o + 1])
            h1 = apool.tile([128, 2 * Bc], F32)
            for o in range(2):
                ps = mmpool.tile([128, 512], F32)
                for k in range(4):
                    nc.tensor.matmul(
                        out=ps[:, :Bc],
                        lhsT=wb1_sb[:, k * 256 + o * 128:k * 256 + o * 128 + 128],
                        rhs=h0[:, k * Bc:(k + 1) * Bc],
                        start=(k == 0), stop=(k == 3))
                nc.scalar.activation(
                    out=h1[:, o * Bc:(o + 1) * Bc], in_=ps[:, :Bc],
                    func=AF.Relu, bias=bb1_sb[:, o:o + 1])
            ps = mmpool.tile([128, 512], F32)
            for k in range(2):
                nc.tensor.matmul(
                    out=ps[:64, :Bc], lhsT=wb2_sb[:, k * 64:(k + 1) * 64],
                    rhs=h1[:, k * Bc:(k + 1) * Bc],
                    start=(k == 0), stop=(k == 1))
            nc.scalar.activation(
                out=featT[0:64, 0:Bc], in_=ps[:64, :Bc],
                func=AF.Relu, bias=bb2_sb[:, 0:1])

            # -------- embedding gather + pool + top-MLP layer 0 --------
            def chunk_matmul(c):
                for o in range(4):
                    nc.tensor.matmul(
                        out=z_ps[o][:, :Bc],
                        lhsT=wt0_sb[:, c * 512 + o * 128:c * 512 + o * 128 + 128],
                        rhs=featT[:, c * Bc:(c + 1) * Bc],
                        start=(c == 0), stop=(c == NCH - 1))

            qn = 0
            for t in range(T):
                c = (t + 1) // 2
                off = 64 * ((t + 1) % 2)
                for j in range(NT):
                    st = spool.tile([128, L, D], BF16, tag="stage")
                    cb = (t * NT + j) * L
                    for l in range(L):
                        _indirect_gather(
                            nc, st[:, l, :], tables.ap(),
                            idx_sb[:, cb + l:cb + l + 1], qn % nq)
                        qn += 1
                    p32 = ppool.tile([128, L // 2, D], F32, tag="p32")
                    nc.vector.tensor_add(
                        out=p32[:, :, :], in0=st[:, 0:10, :],
                        in1=st[:, 10:20, :])
                    nc.vector.tensor_add(
                        out=p32[:, 0:5, :], in0=p32[:, 0:5, :],
                        in1=p32[:, 5:10, :])
                    nc.vector.tensor_add(
                        out=p32[:, 0:2, :], in0=p32[:, 0:2, :],
                        in1=p32[:, 2:4, :])
                    nc.vector.tensor_add(
                        out=p32[:, 0:1, :], in0=p32[:, 0:1, :],
                        in1=p32[:, 1:2, :])
                    nc.vector.tensor_add(
                        out=p32[:, 0:1, :], in0=p32[:, 0:1, :],
                        in1=p32[:, 4:5, :])
                    pst = tppool.tile([64, 128], F32, tag="tp")
                    nc.tensor.transpose(
                        out=pst[:], in_=p32[:, 0, :], identity=ident[:])
                    nc.scalar.copy(
                        out=featT[off:off + 64,
                                  c * Bc + j * 128:c * Bc + (j + 1) * 128],
                        in_=pst[:])
                # feature chunk c complete: chunk 0 after table 0 (+bottom
                # MLP); chunk c>=1 after table 2c; final chunk's upper rows
                # are the zero pad
                if t == 0:
                    chunk_matmul(0)
                elif t % 2 == 0:
                    chunk_matmul(t // 2)
                elif t == T - 1:
                    chunk_matmul((t + 1) // 2)

            # ---------------- top MLP tail ----------------
            z0 = apool.tile([128, 4 * Bc], F32)
            for o in range(4):
                nc.scalar.activation(
                    out=z0[:, o * Bc:(o + 1) * Bc], in_=z_ps[o][:, :Bc],
                    func=AF.Relu, bias=tb0_sb[:, o:o + 1])
            z1 = apool.tile([128, 2 * Bc], F32)
            for o in range(2):
                ps = mmpool.tile([128, 512], F32)
                for k in range(4):
                    nc.tensor.matmul(
                        out=ps[:, :Bc],
                        lhsT=wt1_sb[:, k * 256 + o * 128:k * 256 + o * 128 + 128],
                        rhs=z0[:, k * Bc:(k + 1) * Bc],
                        start=(k == 0), stop=(k == 3))
                nc.scalar.activation(
                    out=z1[:, o * Bc:(o + 1) * Bc], in_=ps[:, :Bc],
                    func=AF.Relu, bias=tb1_sb[:, o:o + 1])
            ps = mmpool.tile([128, 512], F32)
            for k in range(2):
                nc.tensor.matmul(
                    out=ps[0:1, :Bc], lhsT=wt2_sb[:, k:k + 1],
                    rhs=z1[:, k * Bc:(k + 1) * Bc],
                    start=(k == 0), stop=(k == 1))
            ysb = apool.tile([1, Bc], F32)
            nc.scalar.activation(
                out=ysb[:], in_=ps[0:1, :Bc],
                func=AF.Sigmoid, bias=tb2_sb[0:1, 0:1])
            nc.sync.dma_start(out=y.ap(), in_=ysb[:])

    nc.compile()
    return nc


def pack_weights(inp, T=T, D=D):
    NCH = (T + 2) // 2
    f32 = np.float32

    def kchunks(wT, K, M):
        return np.ascontiguousarray(
            wT.reshape(K // 128, 128, M).transpose(1, 0, 2).reshape(128, -1)
        ).astype(f32, copy=False)

    wb0 = np.zeros((128, 512), f32)
    wb0[:DENSE] = inp["bw0"].T
    wb1 = kchunks(np.ascontiguousarray(inp["bw1"].T), 512, 256)
    wb2 = kchunks(np.ascontiguousarray(inp["bw2"].T), 256, 64)
    feat_in = D * (1 + T)
    wt0p = np.zeros((NCH * 128, 512), f32)
    wt0p[:feat_in] = inp["tw0"].T
    wt0 = kchunks(wt0p, NCH * 128, 512)
    wt1 = kchunks(np.ascontiguousarray(inp["tw1"].T), 512, 256)
    wt2 = kchunks(np.ascontiguousarray(inp["tw2"].T), 256, 1)
    return dict(
        wb0=wb0, wb1=wb1, wb2=wb2, wt0=wt0, wt1=wt1, wt2=wt2,
        bb0=np.ascontiguousarray(inp["bb0"].reshape(4, 128).T).astype(f32),
        bb1=np.ascontiguousarray(inp["bb1"].reshape(2, 128).T).astype(f32),
        bb2=inp["bb2"].reshape(64, 1).astype(f32),
        tb0=np.ascontiguousarray(inp["tb0"].reshape(4, 128).T).astype(f32),
        tb1=np.ascontiguousarray(inp["tb1"].reshape(2, 128).T).astype(f32),
        tb2=inp["tb2"].reshape(1, 1).astype(f32),
    )


def pack_core(x_dense, x_indices, c, Bc, NT, T=T, V=V, L=L):
    sl = slice(c * Bc, (c + 1) * Bc)
    xdt = np.zeros((128, Bc), np.float32)
    xdt[:DENSE] = x_dense[sl].T
    idx = x_indices[:, sl, :].astype(np.int32)           # [T, Bc, L]
    idx += (np.arange(T, dtype=np.int32) * V)[:, None, None]
    idxp = np.ascontiguousarray(
        idx.reshape(T, NT, 128, L).transpose(2, 0, 1, 3).reshape(128, T * NT * L)
    )
    return xdt, idxp


_NC_CACHE = {}


def _get_nc():
    if "nc" not in _NC_CACHE:
        _NC_CACHE["nc"] = build_bass()
    return _NC_CACHE["nc"]


def run(inputs, trace=False, **run_kwargs):
    nc = _get_nc()
    NT = 4
    Bc = NT * 128
    shared = pack_weights(inputs)
    tables_flat = np.ascontiguousarray(
        np.asarray(inputs["tables"], dtype=np.float32).reshape(T * V, D)
    ).astype(ml_dtypes.bfloat16)
    x_dense = np.asarray(inputs["x_dense"], dtype=np.float32)
    x_indices = np.asarray(inputs["x_indices"])
    in_maps = []
    for c in range(NCORES):
        xdt, idxp = pack_core(x_dense, x_indices, c, Bc, NT)
        m = dict(shared)
        m["tables"] = tables_flat
        m["xdt"] = xdt
        m["idx"] = idxp
        in_maps.append(m)
    res = run_bass_kernel_spmd(
        nc, in_maps, core_ids=list(range(NCORES)), trace=trace, **run_kwargs)
    yv = np.concatenate([res.results[c]["y"][0] for c in range(NCORES)])
    return yv.reshape(B, 1).astype(np.float32), res


def kernel(**inputs):
    return run(inputs)[0]
